# revision 39
# baseline (speedup 1.0000x reference)
"""Kernel-target-alignment loss on 8 TRN2 NeuronCores.

Math: Xs = X*sqrt(p); d2_ij = ||Xs_i - Xs_j||^2; K = exp(-d2) (diag := 1);
kta = sum(K*tt^T) / (N*sqrt(sum(K*K)));  return -kta.

Strategy (~80 us HW, 3.4x over the first working version):
  * Exact diagonal on host: S2 = N + offdiag, S1 = sum(t^2) + offdiag.
    The device computes only off-diagonal sums; K's diagonal is suppressed
    by adding -BIG to A_ii via a second (identity-weights) matmul on
    diagonal tiles, so no bit-exact sq path is needed and everything runs
    in bf16 (off-diagonal K values are ~1e-4 of the totals, so low
    precision there is harmless).
  * Triangle-of-work: by symmetry only ~half the N^2 pairs are computed.
    Row block r (512 rows) pairs with column blocks r..r+8 (mod 16); core c
    owns row blocks {c, c+8}. Shipping each core its inputs ROLLED left by
    512*c columns makes the tile pattern identical on every core (SPMD):
    rows A = local cols [0,512) x local cts 0..8 (ct0 = diagonal block),
    rows B = local cols [4096,4608) x local cts 8..15 (ct8 = diagonal).
    68 [128,512] half-tiles/core vs 128 for the full matrix.
  * A = 2*G - sq_i - sq_j via one bf16 matmul (K=65: 64 data rows + the
    -sq_j row against an all-ones lhsT row); -sq_i rides the exp bias.
    -sq itself: xb^2 on DVE, then PE one-hot-window matvecs reduce over D
    into [nw,512] PSUM rows; row 64 of R is filled by small SBUF->SBUF
    DMAs and the two bias slices become [128,4] via a PE transpose (no
    partition-scatter DMA on the critical path).
  * Reductions: S2 = sum E^2 via DVE scalar_tensor_tensor+accum per tile
    (a few units instead use ACT Square+accum to balance engines).
    S1 = sum_i t_i E_ij accumulated on the PE: each half-tile issues a
    matvec whose lhsT is a one-hot window holding t for that row block,
    accumulating into one [17,512] PSUM w row per local column tile; a
    single small stt then dots the w rows with t (host applies per-row
    diag/offdiag weights). Software-pipelined with a 2-unit delay so the
    PE never waits on a fresh exp.
"""

import numpy as np
import ml_dtypes

import concourse.bass as bass
import concourse.bacc as bacc
import concourse.tile as tile
import concourse.mybir as mybir
from concourse.bass_utils import run_bass_kernel_spmd

N = 8192
D = 64
NCORES = 8
NB = 16          # 512-row/col blocks
BW = 512         # block width
BIG = 100.0
MULT = mybir.AluOpType.mult

F32 = mybir.dt.float32
BF16 = mybir.dt.bfloat16
BF16NP = ml_dtypes.bfloat16

# unit lists (per rb): (col_start, width, kind); kind: d=diag, w=wide, n=narrow
UNITS_A = [(0, 512, "d"), (512, 1024, "w"), (1536, 1024, "w"),
           (2560, 1024, "w"), (3584, 1024, "w")]
UNITS_B = [(4096, 512, "d"), (4608, 1024, "w"), (5632, 1024, "w"),
           (6656, 1024, "w"), (7680, 512, "n")]


def _ap(tensor, ap, offset=0):
    return bass.AP(tensor=tensor, offset=offset, ap=ap)


def _unit_table():
    """Static flattened unit table, round-robin over row blocks so early
    units touch only the earliest -sq/bias groups."""
    units = []
    uidx = 0
    for rb in range(8):
        for s in range(5):
            is_a = rb < 4
            k = rb % 4
            lcol = 128 * k if is_a else 512 + 128 * k
            a, w, kind = (UNITS_A if is_a else UNITS_B)[s]
            rows = []
            for h in range(w // 512):
                ct = (a + h * 512) // 512
                if kind == "d" and not is_a:
                    ct = 16
                rows.append(ct)
            units.append(dict(
                rb=rb, k=k, lcol=lcol, a=a, w=w, kind=kind,
                uidx=uidx, s2_act=(uidx in (2, 9, 16, 23)), wrows=rows,
                wt=1.0 if kind == "d" else 2.0,
            ))
            uidx += 1
    return units


UNITS = _unit_table()
NUNIT = len(UNITS)           # 40
# host-side weight for each w17 row
WROW_WT = [1.0] + [2.0] * 15 + [1.0]


def build_kernel():
    nc = bacc.Bacc("TRN2", target_bir_lowering=False)

    xb_d = nc.dram_tensor("xb", [D, N], BF16, kind="ExternalInput")
    params_d = nc.dram_tensor("params", [D], F32, kind="ExternalInput")
    zp_d = nc.dram_tensor("zp", [128, 1024], BF16, kind="ExternalInput")
    misc_d = nc.dram_tensor("misc", [128, 532], F32, kind="ExternalInput")
    s1o_d = nc.dram_tensor("s1o", [17], F32, kind="ExternalOutput")
    s2o_d = nc.dram_tensor("s2o", [128, NUNIT], F32, kind="ExternalOutput")

    with tile.TileContext(nc) as tc:
        with (
            tc.tile_pool(name="const", bufs=1) as cpool,
            tc.tile_pool(name="emm", bufs=3, space="PSUM") as mpool,
            tc.tile_pool(name="wps", bufs=1, space="PSUM") as wpool,
            tc.tile_pool(name="etile", bufs=4) as epool,
            tc.tile_pool(name="scr", bufs=2) as spool,
        ):
            # ---- persistent SBUF ----------------------------------------
            R = cpool.tile([D + 1, N], BF16, tag="R")        # [xb ; -sq]
            L = cpool.tile([D + 1, 1024], BF16, tag="L")     # [2p*xb ; ones]
            xb2 = cpool.tile([D, N], BF16, tag="xb2")        # xb*xb
            zp = cpool.tile([128, 1024], BF16, tag="zp")
            misc = cpool.tile([128, 532], F32, tag="misc")
            WT = cpool.tile([128, 8 * 33], BF16, tag="WT")   # t one-hot wins
            NP = cpool.tile([D, 31], BF16, tag="NP")         # -p one-hot win
            trb = cpool.tile([128, 8], BF16, tag="trb")
            biasA = cpool.tile([128, 4], BF16, tag="biasA")
            biasB = cpool.tile([128, 4], BF16, tag="biasB")
            psb = cpool.tile([D, 1], F32, tag="psb")
            p2sb = cpool.tile([D, 1], F32, tag="p2sb")
            npf = cpool.tile([D, 1], F32, tag="npf")
            s2acc = cpool.tile([128, NUNIT], F32, tag="s2acc")
            s1f = cpool.tile([17, 1], F32, tag="s1f")
            wscr = cpool.tile([17, 512], F32, tag="wscr")
            w17 = wpool.tile([17, 512], F32, tag="w17")      # S1 matvec rows

            # ---- input DMAs + pipelined -sq setup -----------------------
            # Queue discipline: sync carries only small latency-critical
            # transfers (params, -sq bounces, bias); R/tcol bulk goes on
            # gpsimd+scalar.
            nc.sync.dma_start(out=psb[:, :], in_=_ap(params_d, [[1, D], [0, 1]]))
            nc.sync.dma_start(out=misc[:, :], in_=misc_d[:, :])
            nc.gpsimd.dma_start(out=R[0:D, 0:512], in_=xb_d[:, 0:512])
            nc.scalar.dma_start(out=R[0:D, 4096:4608], in_=xb_d[:, 4096:4608])
            nc.gpsimd.dma_start(out=R[0:D, 512:2048], in_=xb_d[:, 512:2048])
            nc.scalar.dma_start(out=zp[:, :], in_=zp_d[:, :])
            nc.gpsimd.dma_start(out=R[0:D, 2048:4096], in_=xb_d[:, 2048:4096])
            nc.scalar.dma_start(out=R[0:D, 4608:6144], in_=xb_d[:, 4608:6144])
            nc.scalar.dma_start(out=R[0:D, 6144:8192], in_=xb_d[:, 6144:8192])

            # ---- small setup compute ------------------------------------
            nc.vector.tensor_scalar_mul(p2sb[:, :], psb[:, :], 2.0)
            nc.vector.tensor_scalar_mul(npf[:, :], psb[:, :], -1.0)
            nc.vector.memset(NP[:, :], 0.0)
            nc.vector.tensor_copy(out=NP[:, 15:16], in_=npf[:, :])
            nc.gpsimd.tensor_copy(out=trb[:, :], in_=misc[:, 8:16])
            nc.gpsimd.memset(WT[:, :], 0.0)
            for rb in range(8):
                nc.gpsimd.tensor_copy(out=WT[:, rb * 33 + 16:rb * 33 + 17],
                                      in_=trb[:, rb:rb + 1])
            # L: [2p*xb ; ones] (cols 0..512 = rows A, 512..1024 = rows B)
            nc.gpsimd.memset(L[D:D + 1, :], 1.0)
            nc.vector.tensor_scalar_mul(L[0:D, 0:512], R[0:D, 0:512], p2sb[:, :])
            nc.vector.tensor_scalar_mul(L[0:D, 512:1024], R[0:D, 4096:4608],
                                        p2sb[:, :])

            # ---- -sq, pipelined groups ----------------------------------
            # Bias groups (512 cols) produce -sq as [4,128] via one-hot
            # window matvecs, then PE-transpose -> [128,4] bias (no DMA
            # scatter). Other groups produce [nw,512] rows; row 64 of R is
            # written via small SBUF->SBUF DMAs.
            def sq_group(cols, bias_to=None):
                a, b = cols
                nw = (b - a) // 512
                nc.vector.tensor_tensor(out=xb2[:, a:b], in0=R[0:D, a:b],
                                        in1=R[0:D, a:b], op=MULT)
                qg = wpool.tile([4, 512], F32, tag="qsqg")
                if bias_to is None:
                    for j in range(nw):
                        nc.tensor.matmul(
                            qg[0:nw, :],
                            NP[:, 15 - j:15 + nw - j],
                            xb2[:, a + j * 512:a + (j + 1) * 512],
                            start=(j == 0), stop=(j == nw - 1),
                        )
                    qbt = spool.tile([4, 512], BF16, tag="qb")
                    nc.vector.tensor_copy(out=qbt[0:nw, :], in_=qg[0:nw, :])
                    nc.sync.dma_start(out=R[D:D + 1, a:b], in_=qbt[0:nw, :])
                else:
                    for j in range(4):
                        nc.tensor.matmul(
                            qg[0:4, 0:128],
                            NP[:, 15 - j:19 - j],
                            xb2[:, a + j * 128:a + (j + 1) * 128],
                            start=(j == 0), stop=(j == 3),
                        )
                    qxs = spool.tile([4, 128], F32, tag="qxs")
                    qxb = spool.tile([4, 128], BF16, tag="qxb")
                    bmm = mpool.tile([128, 1024], F32, tag="mm")
                    nc.vector.tensor_copy(out=qxs[:, :], in_=qg[0:4, 0:128])
                    nc.vector.tensor_copy(out=qxb[:, :], in_=qg[0:4, 0:128])
                    nc.sync.dma_start(out=R[D:D + 1, a:b], in_=qxb[:, :])
                    nc.tensor.transpose(bmm[:, 0:4], qxs[:, :],
                                        misc[0:4, 528:532])
                    nc.vector.tensor_copy(out=bias_to[:, :], in_=bmm[:, 0:4])

            sq_group((0, 512), bias_to=biasA)
            sq_group((4096, 4608), bias_to=biasB)
            sq_group((512, 2048))
            sq_group((2048, 4096))

            # ---- main loop ----------------------------------------------
            EXP = mybir.ActivationFunctionType.Exp
            n_mv = sum(len(u["wrows"]) for u in UNITS)
            mv_done = 0
            pending = []  # deferred (per previous unit) DVE/PE reduction ops

            def flush_pending():
                nonlocal mv_done
                for fn in pending:
                    mv_done = fn(mv_done)
                pending.clear()

            for u in UNITS:
                w = u["w"]
                mmt = mpool.tile([128, 1024], F32, tag="mm")
                mm = mmt[:, 0:w]
                for h in range(w // 512):
                    st = True
                    sp = not (u["kind"] == "d" and h == 0)
                    nc.tensor.matmul(
                        mm[:, h * 512:(h + 1) * 512],
                        L[:, u["lcol"]:u["lcol"] + 128],
                        R[:, u["a"] + h * 512:u["a"] + (h + 1) * 512],
                        start=st, stop=sp,
                    )
                if u["kind"] == "d":
                    zoff = 384 - 128 * u["k"]
                    nc.tensor.matmul(
                        mm[:, 0:512], zp[:, 896:1024], zp[:, zoff:zoff + 512],
                        start=False, stop=True,
                    )
                Et = epool.tile([128, 1024], BF16, tag="E")
                E = Et[:, 0:w]
                bt = biasA if u["rb"] < 4 else biasB
                nc.scalar.activation(out=E[:, :], in_=mm[:, :], func=EXP,
                                     bias=bt[:, u["k"]:u["k"] + 1], scale=1.0)

                if len(pending) >= 2:
                    mv_done = pending.pop(0)(mv_done)

                def make_ops(u=u, E=E, w=w):
                    def run(mv_done):
                        sc2t = spool.tile([128, 1024], BF16, tag="sc2")
                        sc2 = sc2t[:, 0:w]
                        if u["s2_act"]:
                            nc.scalar.activation(
                                out=sc2, in_=E[:, :],
                                func=mybir.ActivationFunctionType.Square,
                                accum_out=s2acc[:, u["uidx"]:u["uidx"] + 1],
                            )
                        else:
                            nc.vector.scalar_tensor_tensor(
                                out=sc2, in0=E[:, :], scalar=1.0, in1=E[:, :],
                                op0=MULT, op1=MULT,
                                accum_out=s2acc[:, u["uidx"]:u["uidx"] + 1],
                            )
                        if True:
                            for h, r in enumerate(u["wrows"]):
                                woff = u["rb"] * 33 + 16 - r
                                nc.tensor.matmul(
                                    w17[0:17, :],
                                    WT[:, woff:woff + 17],
                                    E[:, h * 512:(h + 1) * 512],
                                    start=(mv_done == 0),
                                    stop=(mv_done == n_mv - 1),
                                    skip_group_check=True,
                                )
                                mv_done += 1
                        return mv_done
                    return run

                pending.append(make_ops())
                if u["uidx"] == 1:
                    sq_group((4608, 6144))
                elif u["uidx"] == 3:
                    sq_group((6144, 8192))
            flush_pending()
            assert mv_done == n_mv

            # ---- final reductions + output ------------------------------
            nc.vector.scalar_tensor_tensor(
                out=wscr[:, :], in0=w17[0:17, :], scalar=1.0,
                in1=misc[0:17, 16:528],
                op0=MULT, op1=MULT, accum_out=s1f[:, :],
            )
            nc.sync.dma_start(out=_ap(s1o_d, [[1, 17]]), in_=s1f[:, :])
            nc.scalar.dma_start(out=s2o_d[:, :], in_=s2acc[:, :])

    nc.compile()
    return nc


_NC_CACHE = None


def make_in_maps(X, target, params):
    X = np.ascontiguousarray(X, dtype=np.float32)
    target = np.ascontiguousarray(target, dtype=np.float32)
    params = np.ascontiguousarray(params, dtype=np.float32)
    XT = np.ascontiguousarray(X.T)                       # [64, 8192]

    zp = np.zeros((128, 1024), dtype=BF16NP)
    for p in range(128):
        zp[p, 384 + p] = -BIG
        zp[p, 896 + p] = 1.0

    in_maps = []
    for c in range(NCORES):
        XTr = np.roll(XT, -BW * c, axis=1)
        tr = np.roll(target, -BW * c)
        trbv = np.zeros((128, 8), dtype=np.float32)
        trbv[:, 0:4] = tr[0:512].reshape(4, 128).T
        trbv[:, 4:8] = tr[4096:4608].reshape(4, 128).T
        misc = np.zeros((128, 532), dtype=np.float32)
        misc[:, 0:8] = trbv
        misc[:, 8:16] = trbv
        misc[0:16, 16:528] = tr.reshape(16, 512)
        misc[16, 16:528] = tr[4096:4608]
        misc[0:4, 528:532] = np.eye(4, dtype=np.float32)
        in_maps.append({
            "xb": XTr.astype(BF16NP),
            "params": params,
            "zp": zp,
            "misc": misc,
        })
    return in_maps


def kernel(X, target, params):
    global _NC_CACHE
    X = np.ascontiguousarray(X, dtype=np.float32)
    target = np.ascontiguousarray(target, dtype=np.float32)
    params = np.ascontiguousarray(params, dtype=np.float32)

    in_maps = make_in_maps(X, target, params)
    if _NC_CACHE is None:
        _NC_CACHE = build_kernel()
    res = run_bass_kernel_spmd(_NC_CACHE, in_maps, core_ids=list(range(NCORES)))

    s1 = float(np.sum(target.astype(np.float64) ** 2))
    s2 = float(N)
    for c in range(NCORES):
        s1o = res.results[c]["s1o"]      # [17]
        s2o = res.results[c]["s2o"]      # [128, NUNIT]
        for r in range(17):
            s1 += WROW_WT[r] * float(s1o[r])
        for u in UNITS:
            s2 += u["wt"] * float(s2o[:, u["uidx"]].sum())

    val = -s1 / (N * np.sqrt(s2))
    return np.array(val, dtype=np.float32)
